# revision 41
# baseline (speedup 1.0000x reference)
"""Trainium2 Bass kernel for nn_DetectorHelper (seq2seq LSTM anomaly detector).

Architecture: encoder LSTM over T=1024 steps -> decoder LSTM over reversed
sequence emitting a linear projection of the hidden state before each cell
update. Data-parallel over the batch axis: 8 NeuronCores x 16 batch rows.

Per-core design (hidden-quartered col-tiling, fp16 operands, DVE-transposed
recurrent state; 2.0x over the fp32r batch-major baseline: 6.94ms vs 14.0ms
by repeat-differential on HW):
  - Hidden units are split in 4 quarters; col-group q (PE array columns
    32q..32q+31, via tile_position) computes all four gates for quarter q
    as one 256-col fp16 weight-panel stream into PSUM partitions 32q+[0:16].
    The four groups stream concurrently on the 16 32x32 PE subarrays.
  - All matmul operands are fp16 (same PE rate as fp32r, half the SBUF/DMA,
    no round-copy staging, and no fp32r N>=256 restriction); accumulation
    stays fp32 in PSUM.  End-to-end rel err ~3e-4 (gate <= 2e-2).
  - Cell math (2 ACT ops + 3 DVE ops) spans all four partition groups at
    once (gap rows are initialized-garbage lanes, harmless and free).
  - h^T for the next step's stationary operand is produced WITHOUT PE
    transposes (transpose-MM outputs at PSUM partition != 0 fault on HW):
    one DVE 32x32 block-transpose each for sigmoid(o) and tanh(c), then
    hT = soT*tcT directly in fp16.  The block-permuted hidden layout
    (unit u = 64*(p//32)+32*c+(p%32) at partition p, K-chunk c) is folded
    into a host-side row permutation of W_hh / W_out.
  - Gate PSUM ping-pongs between two banks by step parity; next step's
    x-matmuls never wait on this step's activation reads.
"""

import sys

sys.path.insert(0, "/opt/trn_rl_repo")

from contextlib import ExitStack

import numpy as np

B = 16      # batch rows per core
F = 64      # feature dim
H = 256     # hidden dim
HQ = 64     # hidden units per quarter
T = 1024
U = 64      # timesteps per For_i body
XS = 8      # x staging ring slots
N_CORES = 8

_CACHE = {}


def _build(repeat=1, T_=T):
    import concourse.bass as bass
    import concourse.tile as tile
    from concourse import bacc, mybir

    F32 = mybir.dt.float32
    F16 = mybir.dt.float16
    NB = T_ // U

    nc = bacc.Bacc("TRN2", target_bir_lowering=False, debug=False,
                   num_devices=N_CORES)

    xte_d = nc.dram_tensor("xte", [F + 1, T_ * B], F16, kind="ExternalInput").ap()
    xtd_d = nc.dram_tensor("xtd", [F + 1, T_ * B], F16, kind="ExternalInput").ap()
    wih_e_d = nc.dram_tensor("wih_e", [F + 1, 4 * 256], F16, kind="ExternalInput").ap()
    whh_e_d = nc.dram_tensor("whh_e", [128, 8 * 256], F16, kind="ExternalInput").ap()
    wih_d_d = nc.dram_tensor("wih_d", [F + 1, 4 * 256], F16, kind="ExternalInput").ap()
    whh_d_d = nc.dram_tensor("whh_d", [128, 8 * 256], F16, kind="ExternalInput").ap()
    wout_d = nc.dram_tensor("wout", [128, 2 * F], F16, kind="ExternalInput").ap()
    bout_d = nc.dram_tensor("bout", [B, F], F32, kind="ExternalInput").ap()
    out_d = nc.dram_tensor("out", [B, T_ * F], F32, kind="ExternalOutput").ap()

    with tile.TileContext(nc) as tc, ExitStack() as ctx:
        wpool = ctx.enter_context(tc.tile_pool(name="wpool", bufs=1))
        wih_e = wpool.tile([F + 1, 4 * 256], F16, name="wih_e_sb")
        whh_e = wpool.tile([128, 8 * 256], F16, name="whh_e_sb")
        wih_d = wpool.tile([F + 1, 4 * 256], F16, name="wih_d_sb")
        whh_d = wpool.tile([128, 8 * 256], F16, name="whh_d_sb")
        wout = wpool.tile([128, 2 * F], F16, name="wout_sb")
        bout = wpool.tile([B, F], F32, name="bout_sb")
        for sb, dr in [(wih_e, wih_e_d), (whh_e, whh_e_d), (wih_d, wih_d_d),
                       (whh_d, whh_d_d), (wout, wout_d), (bout, bout_d)]:
            nc.sync.dma_start(sb[:], dr[:])

        # persistent state, parity ping-pong: step j reads half p=j%2.
        # hT holds h^T in the DVE 32x32-block-transposed layout: hidden unit
        # u = 64*(p//32) + 32*c + (p%32) lives at partition p, K-chunk c
        # (cols 32c..32c+16 of the parity half); W_hh rows are permuted to
        # match on the host.
        hT = wpool.tile([128, 128], F16, name="hT_sb")
        cst = wpool.tile([128, 2 * HQ], F32, name="c_sb")   # halves [.,64p:]
        xstage = wpool.tile([F + 1, XS * B], F16, name="xstage_sb")

        gpool = ctx.enter_context(tc.tile_pool(name="gpool", bufs=1, space="PSUM"))
        g_ps_ab = [gpool.tile([128, 256], F32, name=f"g_ps_{ab}")
                   for ab in "ab"]
        tpool = ctx.enter_context(tc.tile_pool(name="tpool", bufs=2, space="PSUM"))
        opool = ctx.enter_context(tc.tile_pool(name="opool", bufs=2, space="PSUM"))
        apool = ctx.enter_context(tc.tile_pool(name="apool", bufs=3))
        cpool = ctx.enter_context(tc.tile_pool(name="cpool", bufs=3))
        spool = ctx.enter_context(tc.tile_pool(name="spool", bufs=2))
        kpool = ctx.enter_context(tc.tile_pool(name="kpool", bufs=4))

        # init the psum gate tiles' gap rows once (values are don't-care;
        # keeps ACT reads of the merged partition span on initialized memory)
        for t0 in g_ps_ab:
            nc.vector.memset(t0[:], 0.0)

        TANH = mybir.ActivationFunctionType.Tanh
        SIG = mybir.ActivationFunctionType.Sigmoid
        P4 = 112  # partition span covering the 4 groups (rows 32q+[0:16])

        def step(blk, j, xsrc_d, wih, whh, dec_ostage=None):
            p = j % 2
            hk = [hT[:, 64 * p + 32 * c:64 * p + 32 * c + 16] for c in range(2)]
            c_prev = cst[:, HQ * p:HQ * p + HQ]
            c_next = cst[:, HQ * (1 - p):HQ * (1 - p) + HQ]
            sl = B * (j % XS)
            xslot = xstage[:, sl:sl + B]
            g_ps = g_ps_ab[p]

            nc.sync.dma_start(xslot, xsrc_d[:, bass.ts(blk * U + j, B)])

            # four col-groups stream their gate panels concurrently; x-MMs
            # issued first (no h dependency -> no head-of-line blocking)
            # (skip_group_check: the sim's started-group tracking aliases
            # partition-disjoint groups in one bank; the functional
            # pending-zero model is per-partition and correct)
            for q in range(4):
                nc.tensor.matmul(g_ps[32 * q:32 * q + B, :], xslot,
                                 wih[:, 256 * q:256 * q + 256],
                                 start=True, stop=False, tile_position=(0, 32 * q),
                                 skip_group_check=True)
            for q in range(4):
                o_ap = g_ps[32 * q:32 * q + B, :]
                for c in range(2):
                    nc.tensor.matmul(o_ap, hk[c],
                                     whh[:, (4 * c + q) * 256:(4 * c + q) * 256 + 256],
                                     start=False, stop=(c == 1),
                                     tile_position=(0, 32 * q),
                                     skip_group_check=True)

            if dec_ostage is not None:
                # projection of h BEFORE the update; after the gate MMs so the
                # in-order PE starts the chain-critical h-matmuls first
                ostage, col = dec_ostage
                o_ps = opool.tile([B, F], F32, name="o_ps", tag="o_ps")
                for c in range(2):
                    nc.tensor.matmul(o_ps[:], hk[c], wout[:, F * c:F * c + F],
                                     start=(c == 0), stop=(c == 1))
                nc.vector.tensor_add(ostage[:, col:col + F], o_ps[:], bout[:])

            # tanh(g) and sigmoid(i,f,o): two ACT ops over the merged span
            # (layout per group: [g | i | f | o] x 64); full 128 partitions
            # so the DVE block-transposes below read initialized memory
            tact = apool.tile([128, 256], F32, name="tact")
            nc.scalar.activation(tact[:, 0:64], g_ps[:, 0:64], TANH)
            nc.scalar.activation(tact[:, 64:256], g_ps[:, 64:256], SIG)

            tg = tact[0:P4, 0:64]
            si = tact[0:P4, 64:128]
            sf = tact[0:P4, 128:192]

            fc = cpool.tile([128, HQ], F32, name="fc")
            ig = cpool.tile([128, HQ], F32, name="ig")
            nc.vector.tensor_mul(fc[0:P4, :], sf, c_prev[0:P4, :])
            nc.vector.tensor_mul(ig[0:P4, :], si, tg)
            nc.vector.tensor_add(c_next[0:P4, :], ig[0:P4, :], fc[0:P4, :])
            tch = cpool.tile([128, HQ], F32, name="tch")
            nc.scalar.activation(tch[:], c_next[:], TANH)
            # block-transpose sigmoid(o) on DVE while ACT runs tanh(c)
            soT = cpool.tile([128, HQ], F32, name="soT")
            nc.vector.transpose(soT[:], tact[:, 192:256])

            # block-transpose tanh(c), then hT = sigmoid(o)^T * tanh(c)^T
            # (all in the 32x32-block layout the weights are permuted for)
            tcT = cpool.tile([128, HQ], F32, name="tcT")
            nc.vector.transpose(tcT[:], tch[:])
            nc.vector.tensor_mul(hT[:, 64 * (1 - p):64 * (1 - p) + 64],
                                 soT[:], tcT[:])

        def whole_pass():
            nc.vector.memset(hT[:, 0:64], 0.0)
            nc.vector.memset(cst[:], 0.0)

            with tc.For_i(0, NB) as blk:
                for j in range(U):
                    step(blk, j, xte_d, wih_e, whh_e)

            with tc.For_i(0, NB) as blk:
                ostage = spool.tile([B, U * F], F32, name="ostage")
                for j in range(U):
                    # outputs land reversed within the block (col U-1-j); the
                    # block is stored at t-range [T-(blk+1)U, T-blk*U)
                    step(blk, j, xtd_d, wih_d, whh_d,
                         dec_ostage=(ostage, (U - 1 - j) * F))
                nc.sync.dma_start(out_d[:, bass.ts((NB - 1) - blk, U * F)],
                                  ostage[:])

        if repeat == 1:
            whole_pass()
        else:
            with tc.For_i(0, repeat):
                whole_pass()

    nc.compile()
    return nc


def host_prep(ts_batch, W_ih_enc, W_hh_enc, b_enc, W_ih_dec, W_hh_dec, b_dec,
              W_out, b_out, T_=T):
    # panel column order per quarter q: [g | i | f | o] x 64 hidden units.
    # PyTorch gate-row order in W_*: i[0:256) f[256:512) g[512:768) o[768:1024)
    def panel_cols():
        cols = np.empty(1024, np.int64)
        for q in range(4):
            base = 256 * q
            for s, g0 in enumerate([512, 0, 256, 768]):
                idx = g0 + HQ * q + np.arange(HQ)
                cols[base + 64 * s: base + 64 * s + 64] = idx
        return cols

    COLS = panel_cols()

    # hT block-transposed layout: K-chunk c, partition p holds hidden unit
    # u = 64*(p//32) + 32*c + (p%32)
    P = np.arange(128)
    PERM = [64 * (P // 32) + 32 * c + (P % 32) for c in range(2)]

    def prep_w(W_ih, W_hh, b):
        wihT = np.asarray(W_ih, np.float32).T          # [F, 1024]
        wih_aug = np.concatenate(
            [wihT, np.asarray(b, np.float32)[None, :]], 0)  # [F+1, 1024]
        wih_p = wih_aug[:, COLS].astype(np.float16)
        whhT = np.asarray(W_hh, np.float32).T          # [H, 1024]
        whh_s = whhT[:, COLS]                          # [256, 1024]
        # [128, 8*256]: K-chunk c (row-permuted) x group q at block 4c+q
        whh_p = np.concatenate(
            [whh_s[PERM[c]][:, 256 * q:256 * q + 256]
             for c in range(2) for q in range(4)], 1)   # [128, 2048]
        return np.ascontiguousarray(wih_p), \
            np.ascontiguousarray(whh_p.astype(np.float16))

    wih_e, whh_e = prep_w(W_ih_enc, W_hh_enc, b_enc)
    wih_d, whh_d = prep_w(W_ih_dec, W_hh_dec, b_dec)
    woutT = np.asarray(W_out, np.float32).T            # [256, 64]
    wout_pack = np.ascontiguousarray(np.concatenate(
        [woutT[PERM[c]] for c in range(2)], 1).astype(np.float16))
    bout_b = np.ascontiguousarray(
        np.broadcast_to(np.asarray(b_out, np.float32)[None, :], (B, F)))

    ts = np.asarray(ts_batch, np.float32)[:, :T_]
    in_maps = []
    for d in range(N_CORES):
        tsl = ts[d * B:(d + 1) * B]                       # [16, T_, F]
        xte = np.empty((F + 1, T_ * B), np.float32)
        xte[:F] = tsl.transpose(2, 1, 0).reshape(F, T_ * B)  # col = t*16 + b
        xte[F] = 1.0
        xtd = np.ascontiguousarray(
            xte.reshape(F + 1, T_, B)[:, ::-1, :].reshape(F + 1, T_ * B))
        in_maps.append({
            "xte": np.ascontiguousarray(xte.astype(np.float16)),
            "xtd": xtd.astype(np.float16),
            "wih_e": wih_e, "whh_e": whh_e,
            "wih_d": wih_d, "whh_d": whh_d,
            "wout": wout_pack, "bout": bout_b,
        })
    return in_maps


def kernel(ts_batch, W_ih_enc, W_hh_enc, b_enc, W_ih_dec, W_hh_dec, b_dec,
           W_out, b_out):
    from concourse.bass_utils import run_bass_kernel_spmd

    if "nc" not in _CACHE:
        _CACHE["nc"] = _build()
    nc = _CACHE["nc"]

    in_maps = host_prep(ts_batch, W_ih_enc, W_hh_enc, b_enc, W_ih_dec,
                        W_hh_dec, b_dec, W_out, b_out)
    res = run_bass_kernel_spmd(nc, in_maps, core_ids=list(range(N_CORES)))
    outs = [r["out"].reshape(B, T, F) for r in res.results]
    return np.ascontiguousarray(np.concatenate(outs, 0))


if __name__ == "__main__":
    rng = np.random.default_rng(0)
    demo = {
        "ts_batch": rng.standard_normal((128, T, F), dtype=np.float32),
        "W_ih_enc": rng.standard_normal((1024, F), dtype=np.float32) * 0.06,
        "W_hh_enc": rng.standard_normal((1024, H), dtype=np.float32) * 0.06,
        "b_enc": rng.standard_normal(1024).astype(np.float32) * 0.06,
        "W_ih_dec": rng.standard_normal((1024, F), dtype=np.float32) * 0.06,
        "W_hh_dec": rng.standard_normal((1024, H), dtype=np.float32) * 0.06,
        "b_dec": rng.standard_normal(1024).astype(np.float32) * 0.06,
        "W_out": rng.standard_normal((F, H), dtype=np.float32) * 0.06,
        "b_out": rng.standard_normal(F).astype(np.float32) * 0.06,
    }
    out = kernel(**demo)
    print("kernel output", out.shape, out.dtype, float(np.abs(out).max()))


# revision 42
# speedup vs baseline: 1.1087x; 1.1087x over previous
"""Trainium2 Bass kernel for nn_DetectorHelper (seq2seq LSTM anomaly detector).

Architecture: encoder LSTM over T=1024 steps -> decoder LSTM over reversed
sequence emitting a linear projection of the hidden state before each cell
update. Data-parallel over the batch axis: 8 NeuronCores x 16 batch rows.

Per-core design (hidden-quartered col-tiling, fp16 operands, DVE-transposed
recurrent state; 2.0x over the fp32r batch-major baseline: 6.94ms vs 14.0ms
by repeat-differential on HW):
  - Hidden units are split in 4 quarters; col-group q (PE array columns
    32q..32q+31, via tile_position) computes all four gates for quarter q
    as one 256-col fp16 weight-panel stream into PSUM partitions 32q+[0:16].
    The four groups stream concurrently on the 16 32x32 PE subarrays.
  - All matmul operands are fp16 (same PE rate as fp32r, half the SBUF/DMA,
    no round-copy staging, and no fp32r N>=256 restriction); accumulation
    stays fp32 in PSUM.  End-to-end rel err ~3e-4 (gate <= 2e-2).
  - Cell math (2 ACT ops + 3 DVE ops) spans all four partition groups at
    once (gap rows are initialized-garbage lanes, harmless and free).
  - h^T for the next step's stationary operand is produced WITHOUT PE
    transposes (transpose-MM outputs at PSUM partition != 0 fault on HW):
    one DVE 32x32 block-transpose each for sigmoid(o) and tanh(c), then
    hT = soT*tcT directly in fp16.  The block-permuted hidden layout
    (unit u = 64*(p//32)+32*c+(p%32) at partition p, K-chunk c) is folded
    into a host-side row permutation of W_hh / W_out.
  - Gate PSUM ping-pongs between two banks by step parity; next step's
    x-matmuls never wait on this step's activation reads.
"""

import sys

sys.path.insert(0, "/opt/trn_rl_repo")

from contextlib import ExitStack

import numpy as np

B = 16      # batch rows per core
F = 64      # feature dim
H = 256     # hidden dim
HQ = 64     # hidden units per quarter
T = 1024
U = 64      # timesteps per For_i body
XS = 8      # x staging ring slots
N_CORES = 8

_CACHE = {}


def _build(repeat=1, T_=T):
    import concourse.bass as bass
    import concourse.tile as tile
    from concourse import bacc, mybir

    F32 = mybir.dt.float32
    F16 = mybir.dt.float16
    NB = T_ // U

    nc = bacc.Bacc("TRN2", target_bir_lowering=False, debug=False,
                   num_devices=N_CORES)

    xte_d = nc.dram_tensor("xte", [F + 1, T_ * B], F16, kind="ExternalInput").ap()
    xtd_d = nc.dram_tensor("xtd", [F + 1, T_ * B], F16, kind="ExternalInput").ap()
    wih_e_d = nc.dram_tensor("wih_e", [F + 1, 4 * 256], F16, kind="ExternalInput").ap()
    whh_e_d = nc.dram_tensor("whh_e", [128, 8 * 256], F16, kind="ExternalInput").ap()
    wih_d_d = nc.dram_tensor("wih_d", [F + 1, 4 * 256], F16, kind="ExternalInput").ap()
    whh_d_d = nc.dram_tensor("whh_d", [128, 8 * 256], F16, kind="ExternalInput").ap()
    wout_d = nc.dram_tensor("wout", [128, 2 * F], F16, kind="ExternalInput").ap()
    bout_d = nc.dram_tensor("bout", [B, F], F32, kind="ExternalInput").ap()
    out_d = nc.dram_tensor("out", [B, T_ * F], F32, kind="ExternalOutput").ap()

    with tile.TileContext(nc) as tc, ExitStack() as ctx:
        wpool = ctx.enter_context(tc.tile_pool(name="wpool", bufs=1))
        wih_e = wpool.tile([F + 1, 4 * 256], F16, name="wih_e_sb")
        whh_e = wpool.tile([128, 8 * 256], F16, name="whh_e_sb")
        wih_d = wpool.tile([F + 1, 4 * 256], F16, name="wih_d_sb")
        whh_d = wpool.tile([128, 8 * 256], F16, name="whh_d_sb")
        wout = wpool.tile([128, 2 * F], F16, name="wout_sb")
        bout = wpool.tile([B, F], F32, name="bout_sb")
        for sb, dr in [(wih_e, wih_e_d), (whh_e, whh_e_d), (wih_d, wih_d_d),
                       (whh_d, whh_d_d), (wout, wout_d), (bout, bout_d)]:
            nc.sync.dma_start(sb[:], dr[:])

        # persistent state, parity ping-pong: step j reads half p=j%2.
        # hT holds h^T in the DVE 32x32-block-transposed layout: hidden unit
        # u = 64*(p//32) + 32*c + (p%32) lives at partition p, K-chunk c
        # (cols 32c..32c+16 of the parity half); W_hh rows are permuted to
        # match on the host.
        hT = wpool.tile([128, 128], F16, name="hT_sb")
        cst = wpool.tile([128, 2 * HQ], F32, name="c_sb")   # halves [.,64p:]
        xstage = wpool.tile([F + 1, XS * B], F16, name="xstage_sb")

        gpool = ctx.enter_context(tc.tile_pool(name="gpool", bufs=1, space="PSUM"))
        g_ps_ab = [gpool.tile([128, 256], F32, name=f"g_ps_{ab}")
                   for ab in "ab"]
        tpool = ctx.enter_context(tc.tile_pool(name="tpool", bufs=2, space="PSUM"))
        opool = ctx.enter_context(tc.tile_pool(name="opool", bufs=2, space="PSUM"))
        apool = ctx.enter_context(tc.tile_pool(name="apool", bufs=3))
        cpool = ctx.enter_context(tc.tile_pool(name="cpool", bufs=3))
        spool = ctx.enter_context(tc.tile_pool(name="spool", bufs=2))
        kpool = ctx.enter_context(tc.tile_pool(name="kpool", bufs=4))

        # init the psum gate tiles' gap rows once (values are don't-care;
        # keeps ACT reads of the merged partition span on initialized memory)
        for t0 in g_ps_ab:
            nc.vector.memset(t0[:], 0.0)

        TANH = mybir.ActivationFunctionType.Tanh
        SIG = mybir.ActivationFunctionType.Sigmoid
        P4 = 112  # partition span covering the 4 groups (rows 32q+[0:16])

        def step(blk, j, xsrc_d, wih, whh, dec_ostage=None):
            p = j % 2
            hk = [hT[:, 64 * p + 32 * c:64 * p + 32 * c + 16] for c in range(2)]
            c_prev = cst[:, HQ * p:HQ * p + HQ]
            c_next = cst[:, HQ * (1 - p):HQ * (1 - p) + HQ]
            sl = B * (j % XS)
            xslot = xstage[:, sl:sl + B]
            g_ps = g_ps_ab[p]

            nc.sync.dma_start(xslot, xsrc_d[:, bass.ts(blk * U + j, B)])

            # four col-groups stream their gate panels concurrently; x-MMs
            # issued first (no h dependency -> no head-of-line blocking)
            # (skip_group_check: the sim's started-group tracking aliases
            # partition-disjoint groups in one bank; the functional
            # pending-zero model is per-partition and correct)
            for q in range(4):
                nc.tensor.matmul(g_ps[32 * q:32 * q + B, :], xslot,
                                 wih[:, 256 * q:256 * q + 256],
                                 start=True, stop=False, tile_position=(0, 32 * q),
                                 skip_group_check=True)
            for q in range(4):
                o_ap = g_ps[32 * q:32 * q + B, :]
                for c in range(2):
                    nc.tensor.matmul(o_ap, hk[c],
                                     whh[:, (4 * c + q) * 256:(4 * c + q) * 256 + 256],
                                     start=False, stop=(c == 1),
                                     tile_position=(0, 32 * q),
                                     skip_group_check=True)

            if dec_ostage is not None:
                # projection of h BEFORE the update; after the gate MMs so the
                # in-order PE starts the chain-critical h-matmuls first
                ostage, col = dec_ostage
                o_ps = opool.tile([B, F], F32, name="o_ps", tag="o_ps")
                for c in range(2):
                    nc.tensor.matmul(o_ps[:], hk[c], wout[:, F * c:F * c + F],
                                     start=(c == 0), stop=(c == 1))
                nc.vector.tensor_add(ostage[:, col:col + F], o_ps[:], bout[:])

            # tanh(g) and sigmoid(i,f,o): two ACT ops over the merged span
            # (layout per group: [g | i | f | o] x 64); full 128 partitions
            # so the DVE block-transposes below read initialized memory
            tact = apool.tile([128, 256], F32, name="tact")
            # sigmoid(i,f) first: it alone gates the DVE fc-mul; tanh(g)
            # second (gates ig); sigmoid(o) last, fully off the chain (only
            # feeds the soT block-transpose that runs during tanh(c))
            nc.scalar.activation(tact[:, 64:192], g_ps[:, 64:192], SIG)
            nc.scalar.activation(tact[:, 0:64], g_ps[:, 0:64], TANH)
            nc.scalar.activation(tact[:, 192:256], g_ps[:, 192:256], SIG)

            tg = tact[0:P4, 0:64]
            si = tact[0:P4, 64:128]
            sf = tact[0:P4, 128:192]

            fc = cpool.tile([128, HQ], F32, name="fc")
            ig = cpool.tile([128, HQ], F32, name="ig")
            nc.vector.tensor_mul(fc[0:P4, :], sf, c_prev[0:P4, :])
            nc.vector.tensor_mul(ig[0:P4, :], si, tg)
            nc.vector.tensor_add(c_next[0:P4, :], ig[0:P4, :], fc[0:P4, :])
            tch = cpool.tile([128, HQ], F32, name="tch")
            nc.scalar.activation(tch[:], c_next[:], TANH)
            # block-transpose sigmoid(o) on DVE while ACT runs tanh(c)
            soT = cpool.tile([128, HQ], F32, name="soT")
            nc.vector.transpose(soT[:], tact[:, 192:256])

            # block-transpose tanh(c), then hT = sigmoid(o)^T * tanh(c)^T
            # (all in the 32x32-block layout the weights are permuted for)
            tcT = cpool.tile([128, HQ], F32, name="tcT")
            nc.vector.transpose(tcT[:], tch[:])
            nc.vector.tensor_mul(hT[:, 64 * (1 - p):64 * (1 - p) + 64],
                                 soT[:], tcT[:])

        def whole_pass():
            nc.vector.memset(hT[:, 0:64], 0.0)
            nc.vector.memset(cst[:], 0.0)

            with tc.For_i(0, NB) as blk:
                for j in range(U):
                    step(blk, j, xte_d, wih_e, whh_e)

            with tc.For_i(0, NB) as blk:
                ostage = spool.tile([B, U * F], F32, name="ostage")
                for j in range(U):
                    # outputs land reversed within the block (col U-1-j); the
                    # block is stored at t-range [T-(blk+1)U, T-blk*U)
                    step(blk, j, xtd_d, wih_d, whh_d,
                         dec_ostage=(ostage, (U - 1 - j) * F))
                nc.sync.dma_start(out_d[:, bass.ts((NB - 1) - blk, U * F)],
                                  ostage[:])

        if repeat == 1:
            whole_pass()
        else:
            with tc.For_i(0, repeat):
                whole_pass()

    nc.compile()
    return nc


def host_prep(ts_batch, W_ih_enc, W_hh_enc, b_enc, W_ih_dec, W_hh_dec, b_dec,
              W_out, b_out, T_=T):
    # panel column order per quarter q: [g | i | f | o] x 64 hidden units.
    # PyTorch gate-row order in W_*: i[0:256) f[256:512) g[512:768) o[768:1024)
    def panel_cols():
        cols = np.empty(1024, np.int64)
        for q in range(4):
            base = 256 * q
            for s, g0 in enumerate([512, 0, 256, 768]):
                idx = g0 + HQ * q + np.arange(HQ)
                cols[base + 64 * s: base + 64 * s + 64] = idx
        return cols

    COLS = panel_cols()

    # hT block-transposed layout: K-chunk c, partition p holds hidden unit
    # u = 64*(p//32) + 32*c + (p%32)
    P = np.arange(128)
    PERM = [64 * (P // 32) + 32 * c + (P % 32) for c in range(2)]

    def prep_w(W_ih, W_hh, b):
        wihT = np.asarray(W_ih, np.float32).T          # [F, 1024]
        wih_aug = np.concatenate(
            [wihT, np.asarray(b, np.float32)[None, :]], 0)  # [F+1, 1024]
        wih_p = wih_aug[:, COLS].astype(np.float16)
        whhT = np.asarray(W_hh, np.float32).T          # [H, 1024]
        whh_s = whhT[:, COLS]                          # [256, 1024]
        # [128, 8*256]: K-chunk c (row-permuted) x group q at block 4c+q
        whh_p = np.concatenate(
            [whh_s[PERM[c]][:, 256 * q:256 * q + 256]
             for c in range(2) for q in range(4)], 1)   # [128, 2048]
        return np.ascontiguousarray(wih_p), \
            np.ascontiguousarray(whh_p.astype(np.float16))

    wih_e, whh_e = prep_w(W_ih_enc, W_hh_enc, b_enc)
    wih_d, whh_d = prep_w(W_ih_dec, W_hh_dec, b_dec)
    woutT = np.asarray(W_out, np.float32).T            # [256, 64]
    wout_pack = np.ascontiguousarray(np.concatenate(
        [woutT[PERM[c]] for c in range(2)], 1).astype(np.float16))
    bout_b = np.ascontiguousarray(
        np.broadcast_to(np.asarray(b_out, np.float32)[None, :], (B, F)))

    ts = np.asarray(ts_batch, np.float32)[:, :T_]
    in_maps = []
    for d in range(N_CORES):
        tsl = ts[d * B:(d + 1) * B]                       # [16, T_, F]
        xte = np.empty((F + 1, T_ * B), np.float32)
        xte[:F] = tsl.transpose(2, 1, 0).reshape(F, T_ * B)  # col = t*16 + b
        xte[F] = 1.0
        xtd = np.ascontiguousarray(
            xte.reshape(F + 1, T_, B)[:, ::-1, :].reshape(F + 1, T_ * B))
        in_maps.append({
            "xte": np.ascontiguousarray(xte.astype(np.float16)),
            "xtd": xtd.astype(np.float16),
            "wih_e": wih_e, "whh_e": whh_e,
            "wih_d": wih_d, "whh_d": whh_d,
            "wout": wout_pack, "bout": bout_b,
        })
    return in_maps


def kernel(ts_batch, W_ih_enc, W_hh_enc, b_enc, W_ih_dec, W_hh_dec, b_dec,
           W_out, b_out):
    from concourse.bass_utils import run_bass_kernel_spmd

    if "nc" not in _CACHE:
        _CACHE["nc"] = _build()
    nc = _CACHE["nc"]

    in_maps = host_prep(ts_batch, W_ih_enc, W_hh_enc, b_enc, W_ih_dec,
                        W_hh_dec, b_dec, W_out, b_out)
    res = run_bass_kernel_spmd(nc, in_maps, core_ids=list(range(N_CORES)))
    outs = [r["out"].reshape(B, T, F) for r in res.results]
    return np.ascontiguousarray(np.concatenate(outs, 0))


if __name__ == "__main__":
    rng = np.random.default_rng(0)
    demo = {
        "ts_batch": rng.standard_normal((128, T, F), dtype=np.float32),
        "W_ih_enc": rng.standard_normal((1024, F), dtype=np.float32) * 0.06,
        "W_hh_enc": rng.standard_normal((1024, H), dtype=np.float32) * 0.06,
        "b_enc": rng.standard_normal(1024).astype(np.float32) * 0.06,
        "W_ih_dec": rng.standard_normal((1024, F), dtype=np.float32) * 0.06,
        "W_hh_dec": rng.standard_normal((1024, H), dtype=np.float32) * 0.06,
        "b_dec": rng.standard_normal(1024).astype(np.float32) * 0.06,
        "W_out": rng.standard_normal((F, H), dtype=np.float32) * 0.06,
        "b_out": rng.standard_normal(F).astype(np.float32) * 0.06,
    }
    out = kernel(**demo)
    print("kernel output", out.shape, out.dtype, float(np.abs(out).max()))
